# revision 1
# baseline (speedup 1.0000x reference)
"""CenterLoss kernel for Trainium2 (Bass/Tile), data-parallel over 8 NeuronCores.

reference:
    d_i = ||x_i||^2 + ||centers[l_i]||^2 - 2 x_i . centers[l_i]   (= ||x_i - c_{l_i}||^2)
    loss = mean_i clip(d_i, 1e-12, 1e12)

Only the label-gathered entry of the [N, C] distance matrix is used, so the
kernel never forms it: each core gathers centers[labels] with the Q7
dma_gather extended instruction (2048 rows per instruction), computes
(x - c)^2 via DVE subtract + ACT square-with-accumulate, reduces to a scalar
partial sum, and the host combines the 8 partials into the mean.
The clip is a provable no-op for this input distribution (d_i ~ chi^2-like,
concentrated around 256; min over N is >> 1e-12).

Sharding: x/labels split into 8 contiguous row shards; centers replicated.

Layouts per core (ROWS=8192 rows, D=128):
  x tile, chunk c: [128, 16*128] f32, partition p holds rows c*2048 + p*16 .. +15
                   (8 KiB contiguous per partition -> efficient DMA)
  gather, chunk c: dma_gather dst[i%128, i//128, :] = centers[idx_i], so host
                   orders idx_i = labels[c*2048 + (i%128)*16 + (i//128)] to
                   match the x layout. Indices int16, wrapped over 16
                   partitions: idxs[i%16, c*128 + i//16].
"""

import numpy as np

import concourse.bacc as bacc
import concourse.bass as bass
import concourse.tile as tile
from concourse import mybir
from concourse.bass_utils import run_bass_kernel_spmd
from concourse.library_config import mlp

N, C, D = 65536, 1000, 128
N_CORES = 8
P = 128
ROWS_PER_CORE = N // N_CORES            # 8192
CHUNK_ROWS = 512                        # rows gathered/processed per chunk
NCHUNK = ROWS_PER_CORE // CHUNK_ROWS    # 16
SUB = CHUNK_ROWS // P                   # 16 rows per partition per chunk
IDXCOLS = CHUNK_ROWS // 16              # 128 idx columns per chunk

_NC = None


def _build_nc():
    f32 = mybir.dt.float32
    nc = bacc.Bacc(trn_type="TRN2", num_swdge_queues=4, dynamic_dma_scratch_size=65536)

    x = nc.dram_tensor("x", [ROWS_PER_CORE, D], f32, kind="ExternalInput")
    idx16 = nc.dram_tensor(
        "idx16", [P, NCHUNK * IDXCOLS], mybir.dt.int16, kind="ExternalInput"
    )
    centers = nc.dram_tensor("centers", [C, D], f32, kind="ExternalInput")
    out = nc.dram_tensor("out", [1, 1], f32, kind="ExternalOutput")

    # [NCHUNK, P, SUB*D]; partition p of chunk c holds rows c*2048 + p*16 .. +15
    x_r = x.ap().rearrange("(c p s) d -> c p (s d)", p=P, s=SUB)

    with tile.TileContext(nc) as tc:
        with (
            tc.tile_pool(name="xp", bufs=16) as xp,
            tc.tile_pool(name="cp", bufs=16) as cp,
            tc.tile_pool(name="small", bufs=1) as small,
            tc.tile_pool(name="psp", bufs=1, space="PSUM") as psp,
        ):
            # eager Q7 library load so the first gather doesn't stall on the
            # lazy IRAM code fetch
            nc.gpsimd.load_library(mlp)

            idx = small.tile([P, NCHUNK * IDXCOLS], mybir.dt.int16)
            nc.sync.dma_start(out=idx[:], in_=idx16.ap())

            acc = small.tile([P, NCHUNK], f32)
            # queues 1-3 generate descriptors on background Q7 workers; queue 0
            # generates inline on the Pool engine (a 4th worker) while the
            # background queues churn. Small chunks start data drains early.
            # queues 1-3 run on background Q7 workers; queue 0 generates inline
            # on the engine. Each period: 6 background enqueues, then 2 inline
            # gens (workers churn while the engine generates). The period of 8
            # matches the 8 DMASW sem lanes so lanes stay queue-consistent.
            QUEUE = [1, 2, 3, 0] * 4
            xts, cts = {}, {}
            for c in range(NCHUNK):
                xt = xp.tile([P, SUB * D], f32, tag="xt")
                nc.sync.dma_start(out=xt[:], in_=x_r[c])
                ct = cp.tile([P, SUB * D], f32, tag="ct")
                nc.gpsimd.dma_gather(
                    ct[:].rearrange("p (s d) -> p s d", s=SUB),
                    centers.ap(),
                    idx[:, c * IDXCOLS:(c + 1) * IDXCOLS],
                    CHUNK_ROWS,
                    CHUNK_ROWS,
                    D,
                    queue_num=QUEUE[c],
                    single_packet=False,
                )
                xts[c], cts[c] = xt, ct
            for c in range(NCHUNK):
                xt, ct = xts[c], cts[c]
                nc.vector.tensor_tensor(
                    out=xt[:], in0=xt[:], in1=ct[:], op=mybir.AluOpType.subtract
                )
                nc.scalar.activation(
                    out=xt[:],
                    in_=xt[:],
                    func=mybir.ActivationFunctionType.Square,
                    accum_out=acc[:, c:c + 1],
                )

            dsum = small.tile([P, 1], f32)
            nc.vector.tensor_reduce(
                out=dsum[:], in_=acc[:], axis=mybir.AxisListType.X,
                op=mybir.AluOpType.add,
            )
            ones = small.tile([P, 1], f32)
            nc.vector.memset(ones[:], 1.0)
            ps = psp.tile([1, 1], f32)
            nc.tensor.matmul(out=ps[:], lhsT=ones[:], rhs=dsum[:], start=True, stop=True)
            res = small.tile([1, 1], f32)
            nc.vector.tensor_copy(out=res[:], in_=ps[:])
            nc.sync.dma_start(out=out.ap(), in_=res[:])

    nc.compile()
    return nc


def _get_nc():
    global _NC
    if _NC is None:
        _NC = _build_nc()
    return _NC


def _make_idx16(lab_core):
    """Wrap one core's labels into the dma_gather int16 index layout."""
    idx16 = np.zeros((16, NCHUNK * IDXCOLS), dtype=np.int16)
    i = np.arange(CHUNK_ROWS)
    for c in range(NCHUNK):
        vals = lab_core[c * CHUNK_ROWS + (i % P) * SUB + (i // P)]
        idx16[i % 16, c * IDXCOLS + i // 16] = vals.astype(np.int16)
    # the 8 Q7 cores each read their own 16-partition replica of the indices
    return np.ascontiguousarray(np.tile(idx16, (8, 1)))


def make_in_maps(x, labels, centers):
    x = np.ascontiguousarray(np.asarray(x), dtype=np.float32)
    labels_np = np.asarray(labels).astype(np.int64)
    centers = np.ascontiguousarray(np.asarray(centers), dtype=np.float32)
    in_maps = []
    for m in range(N_CORES):
        lo = m * ROWS_PER_CORE
        in_maps.append({
            "x": x[lo:lo + ROWS_PER_CORE],
            "idx16": _make_idx16(labels_np[lo:lo + ROWS_PER_CORE]),
            "centers": centers,
        })
    return in_maps


def run(x, labels, centers, **spmd_kwargs):
    """Run on the 8 NeuronCores; returns (loss, BassKernelResults)."""
    nc = _get_nc()
    in_maps = make_in_maps(x, labels, centers)
    res = run_bass_kernel_spmd(nc, in_maps, core_ids=list(range(N_CORES)), **spmd_kwargs)
    total = sum(float(r["out"][0, 0]) for r in res.results)
    return np.float32(total / N), res


def kernel(x, labels, centers):
    loss, _ = run(x, labels, centers)
    return loss



# revision 6
# speedup vs baseline: 1.0186x; 1.0186x over previous
"""CenterLoss kernel for Trainium2 (Bass/Tile), data-parallel over 8 NeuronCores.

reference:
    d_i = ||x_i||^2 + ||centers[l_i]||^2 - 2 x_i . centers[l_i]   (= ||x_i - c_{l_i}||^2)
    loss = mean_i clip(d_i, 1e-12, 1e12)

Only the label-gathered entry of the [N, C] distance matrix is used, so the
kernel never forms it: each core gathers centers[labels] with the Q7
dma_gather extended instruction, computes (x - c)^2 via DVE subtract + ACT
square-with-accumulate, reduces to a scalar partial sum, and the host
combines the 8 partials into the mean.  The clip is a no-op for this input
distribution (d_i concentrated around 256).

Performance structure (vs the naive version):
  * data moves at reduced precision (bf16 or fp8-e4m3); the mean absorbs the
    rounding noise (measured ~4e-6 rel for bf16, ~8e-4 for fp8, gate 2e-2).
  * rows are HOST-SORTED by label and grouped K to a gather index: one Q7
    descriptor then covers K rows (source = centers_rep, each row replicated
    K times).  SWDGE descriptor generation is the dominant wall (~9ns/idx per
    worker, only 4 generation streams), so K divides it.
  * class segments are padded to a multiple of K with zero-rows pointing at a
    zeros row of centers_rep -> pads contribute exactly 0 to the sum.
  * gathers are issued FIRST (background SWDGE queues 1-3, inline queue 0
    last) and x streams via a few large DMAs so the two transfer phases and
    compute fully overlap.

Sharding: x/labels split into 8 contiguous row shards; centers replicated.
"""

import numpy as np
import ml_dtypes

import concourse.bacc as bacc
import concourse.bass as bass
import concourse.tile as tile
from concourse import mybir
from concourse.bass_utils import run_bass_kernel_spmd
from concourse.library_config import mlp

N, C, D = 65536, 1000, 128
N_CORES = 8
P = 128
ROWS_PER_CORE = N // N_CORES            # 8192

# --- tunables -------------------------------------------------------------
DTYPE = "bf16"          # "bf16" | "fp8"
K = 1                   # rows per gather index (host sorts + pads classes)
NCHUNK = 8              # compute/gather chunks
X_DMAS = 4              # number of x dma_start instructions
SINGLE_PACKET = True
# queue per chunk; 1..3 = background SWDGE workers, 0 = inline on Pool engine
QUEUES = [1, 2, 3, 1, 2, 3, 0, 0]

_NP_DT = {"bf16": ml_dtypes.bfloat16, "fp8": ml_dtypes.float8_e4m3fn}
_MY_DT = {"bf16": mybir.dt.bfloat16, "fp8": mybir.dt.float8e4}

_NC_CACHE = {}


def _plan(labels_np):
    """Padded-rows plan shared by all cores (single SPMD program)."""
    if K == 1:
        rows_p = ROWS_PER_CORE
    else:
        need = 0
        for m in range(N_CORES):
            cnt = np.bincount(labels_np[m * ROWS_PER_CORE:(m + 1) * ROWS_PER_CORE],
                              minlength=C)
            need = max(need, ROWS_PER_CORE + int(((-cnt) % K).sum()))
        # rows per chunk must be a multiple of 128*K
        rows_p = -(-need // (NCHUNK * 128 * K)) * (NCHUNK * 128 * K)
    chunk = rows_p // NCHUNK
    return rows_p, chunk


def _build_nc(rows_p, chunk):
    f32 = mybir.dt.float32
    dt = _MY_DT[DTYPE]
    gc = chunk // K                 # gather indices per chunk
    s2 = gc // P                    # group slots per partition per chunk
    cols = chunk                    # elements per partition per chunk (s2*K*D/P*... = chunk)
    icols = gc // 16                # idx columns per chunk

    nc = bacc.Bacc(trn_type="TRN2", num_swdge_queues=4, dynamic_dma_scratch_size=65536)

    x = nc.dram_tensor("x", [NCHUNK, P, cols], dt, kind="ExternalInput")
    idx16 = nc.dram_tensor("idx16", [P, NCHUNK * icols], mybir.dt.int16,
                           kind="ExternalInput")
    centers_rep = nc.dram_tensor("centers_rep", [C + 1, K * D], dt,
                                 kind="ExternalInput")
    out = nc.dram_tensor("out", [1, 1], f32, kind="ExternalOutput")

    with tile.TileContext(nc) as tc:
        with (
            tc.tile_pool(name="cp", bufs=NCHUNK) as cp,
            tc.tile_pool(name="small", bufs=1) as small,
            tc.tile_pool(name="psp", bufs=1, space="PSUM") as psp,
        ):
            # eager Q7 library load so the first gather doesn't stall on the
            # lazy IRAM code fetch
            nc.gpsimd.load_library(mlp)

            idx = small.tile([P, NCHUNK * icols], mybir.dt.int16)
            nc.sync.dma_start(out=idx[:], in_=idx16.ap())

            acc = small.tile([P, NCHUNK], f32)
            xts = [None] * NCHUNK
            cts = [cp.tile([P, cols], dt, tag="ct", name=f"ct{c}")
                   for c in range(NCHUNK)]

            def emit_gather(c):
                nc.gpsimd.dma_gather(
                    cts[c][:].rearrange("p (s e) -> p s e", s=s2),
                    centers_rep.ap(),
                    idx[:, c * icols:(c + 1) * icols],
                    gc, gc, K * D,
                    queue_num=QUEUES[c],
                    single_packet=SINGLE_PACKET,
                )

            # background-queue gathers first (cheap ring enqueues), inline last
            for c in range(NCHUNK):
                if QUEUES[c] != 0:
                    emit_gather(c)

            # x loads: a few large DMAs (each dma_start costs ~600ns of Sync)
            per = NCHUNK // X_DMAS
            for g in range(X_DMAS):
                dst = small.tile([P, per * cols], dt, tag=f"xg{g}")
                nc.sync.dma_start(
                    out=dst[:].rearrange("p (c f) -> p c f", c=per),
                    in_=x.ap()[g * per:(g + 1) * per].rearrange("c p f -> p c f"),
                )
                for c in range(g * per, (g + 1) * per):
                    xts[c] = dst[:, (c - g * per) * cols:(c - g * per + 1) * cols]

            for c in range(NCHUNK):
                if QUEUES[c] == 0:
                    emit_gather(c)

            for c in range(NCHUNK):
                xt, ct = xts[c], cts[c]
                nc.vector.tensor_tensor(out=xt, in0=xt, in1=ct[:],
                                        op=mybir.AluOpType.subtract)
                nc.scalar.activation(
                    out=xt, in_=xt,
                    func=mybir.ActivationFunctionType.Square,
                    accum_out=acc[:, c:c + 1],
                )

            dsum = small.tile([P, 1], f32)
            nc.vector.tensor_reduce(out=dsum[:], in_=acc[:], axis=mybir.AxisListType.X,
                                    op=mybir.AluOpType.add)
            ones = small.tile([P, 1], f32)
            nc.vector.memset(ones[:], 1.0)
            ps = psp.tile([1, 1], f32)
            nc.tensor.matmul(out=ps[:], lhsT=ones[:], rhs=dsum[:], start=True, stop=True)
            res = small.tile([1, 1], f32)
            nc.vector.tensor_copy(out=res[:], in_=ps[:])
            nc.sync.dma_start(out=out.ap(), in_=res[:])

    nc.compile()
    return nc


def _get_nc(rows_p, chunk):
    key = (DTYPE, K, NCHUNK, X_DMAS, SINGLE_PACKET, tuple(QUEUES), rows_p)
    if key not in _NC_CACHE:
        _NC_CACHE[key] = _build_nc(rows_p, chunk)
    return _NC_CACHE[key]


def _core_inputs(x_core, lab_core, rows_p, chunk):
    """Sort rows by label, pad class segments to K, lay out x in device order
    and build the wrapped int16 gather indices."""
    np_dt = _NP_DT[DTYPE]
    gc = chunk // K
    s2 = gc // P
    icols = gc // 16
    total_g = rows_p // K

    order = np.argsort(lab_core, kind="stable")
    slab = lab_core[order]
    if K == 1:
        g_of_row = np.arange(ROWS_PER_CORE)
        g_class = slab.astype(np.int16)
        n_groups = ROWS_PER_CORE
    else:
        cnt = np.bincount(slab, minlength=C)
        pad = (-cnt) % K
        gcnt = (cnt + pad) // K
        n_groups = int(gcnt.sum())
        # class of each real group, in sorted-class order
        g_class_real = np.repeat(np.arange(C), gcnt).astype(np.int16)
        # group id of each sorted row: per-class base + within-class offset
        gbase = np.concatenate([[0], np.cumsum(gcnt)[:-1]])
        within = np.arange(ROWS_PER_CORE) - np.repeat(
            np.concatenate([[0], np.cumsum(cnt)[:-1]]), cnt)
        g_of_row = gbase[slab] * K + within  # padded-row index of each sorted row
        g_class = np.full(total_g, C, dtype=np.int16)
        g_class[:n_groups] = g_class_real

    # device x layout: group g -> chunk g//gc, partition g%P... (see gather
    # doc: gathered index i lands on partition i%128, slot i//128)
    x_dev = np.zeros((NCHUNK, P, s2, K, D), dtype=np_dt)
    g = g_of_row if K > 1 else np.arange(ROWS_PER_CORE)
    grp = g // K
    k_off = g % K
    c_i = grp // gc
    g_loc = grp % gc
    p_i = g_loc % P
    s_i = g_loc // P
    x_dev[c_i, p_i, s_i, k_off, :] = x_core[order].astype(np_dt)

    idx16 = np.zeros((16, NCHUNK * icols), dtype=np.int16)
    gg = np.arange(total_g)
    idx16[(gg % gc) % 16, (gg // gc) * icols + (gg % gc) // 16] = g_class
    return (x_dev.reshape(NCHUNK, P, chunk),
            np.ascontiguousarray(np.tile(idx16, (8, 1))))


def make_in_maps(x, labels, centers, rows_p, chunk):
    x = np.ascontiguousarray(np.asarray(x), dtype=np.float32)
    labels_np = np.asarray(labels).astype(np.int64)
    centers = np.asarray(centers).astype(np.float32)
    crep = np.zeros((C + 1, K * D), dtype=_NP_DT[DTYPE])
    crep[:C] = np.tile(centers, (1, K)).astype(_NP_DT[DTYPE])
    in_maps = []
    for m in range(N_CORES):
        lo = m * ROWS_PER_CORE
        x_dev, idx16 = _core_inputs(x[lo:lo + ROWS_PER_CORE],
                                    labels_np[lo:lo + ROWS_PER_CORE], rows_p, chunk)
        in_maps.append({"x": x_dev, "idx16": idx16, "centers_rep": crep})
    return in_maps


def run(x, labels, centers, **spmd_kwargs):
    """Run on the 8 NeuronCores; returns (loss, BassKernelResults)."""
    labels_np = np.asarray(labels).astype(np.int64)
    rows_p, chunk = _plan(labels_np)
    nc = _get_nc(rows_p, chunk)
    in_maps = make_in_maps(x, labels_np, centers, rows_p, chunk)
    res = run_bass_kernel_spmd(nc, in_maps, core_ids=list(range(N_CORES)), **spmd_kwargs)
    total = sum(float(r["out"][0, 0]) for r in res.results)
    return np.float32(total / N), res


def kernel(x, labels, centers):
    loss, _ = run(x, labels, centers)
    return loss
